# revision 30
# baseline (speedup 1.0000x reference)
"""MLPConv (3x3 valid conv -> 256 -> 256 MLP with ReLU) on 8 TRN2 cores.

Data-parallel over batch (4 images/core). The host pre-transposes each
image to [C=128, pixels] bf16 so the device runs a pure matmul stream
(no transposes/casts). Warmup matmuls on a memset tile flip the HAM
clock gate to 2.4 GHz during the input DMA; critical prologue DMAs are
split across both HWDGE rings with non-critical prefetches held back by
real dependencies. The moving operand streams 2-D blocks of [rows, 62
valid cols] via a 3-D access pattern so the 64-wide grid's 2 garbage
columns are never computed. Stage-2 post-processing alternates scalar
ACT and DVE; outputs go out as bf16 (host upcasts).

Blocks: 7x(8 rows -> N=496) + 1x(6 rows -> N=372) = 62 rows.
Output per core is [2, 128, img, 3844] = [F, img, 62, 62] directly.
"""

import numpy as np
import ml_dtypes

import concourse.bass as bass
import concourse.mybir as mybir
import concourse.tile as tile
from concourse.bass_utils import run_bass_kernel_spmd

B, H, W, C = 32, 64, 64, 128
F = 256
N_CORES = 8
IMG_PER_CORE = B // N_CORES
HW = H * W
GRID = 62 * 62                  # 3844 valid output pixels
NBLK = 8
ROWS = [8] * 7 + [6]            # output rows per block
BLKN = [r * 62 for r in ROWS]   # 496 x7, 372
BLKS = [496 * i for i in range(8)]  # start col in compact grid
XCOLS = HW

F32 = mybir.dt.float32
BF16 = mybir.dt.bfloat16
RELU = mybir.ActivationFunctionType.Relu


def _split_multi_waits(nc):
    """This container's walrus rejects >1 semaphore wait per instruction
    ("Too many sync wait commands"). Move all but the last wait of each
    instruction onto single-wait NoOps right before it on the same engine."""
    n = 0
    for f in nc.m.functions:
        for bb in f.blocks:
            insts = bb.instructions
            if not any(
                i.sync_info is not None and len(i.sync_info.on_wait) > 1
                for i in insts
            ):
                continue
            new_insts = []
            for inst in insts:
                si = inst.sync_info
                if si is not None and len(si.on_wait) > 1:
                    waits = list(si.on_wait)
                    for k, w in enumerate(waits[:-1]):
                        new_insts.append(
                            mybir.InstNoOp(
                                name=f"{inst.name}-wsplit{k}",
                                engine=inst.engine,
                                bass_nofuse=True,
                                sync_info=mybir.SyncInfo(on_wait=[w], on_update=[]),
                            )
                        )
                        n += 1
                    inst.sync_info = mybir.SyncInfo(
                        on_wait=[waits[-1]], on_update=list(si.on_update)
                    )
                new_insts.append(inst)
            bb.instructions = new_insts
    return n


def build_nc():
    nc = bass.Bass("TRN2", target_bir_lowering=False)
    x = nc.dram_tensor(
        "x", [IMG_PER_CORE, C, XCOLS], BF16, kind="ExternalInput"
    ).ap()
    w0 = nc.dram_tensor("w0", [C, 9, F], BF16, kind="ExternalInput").ap()
    w1 = nc.dram_tensor("w1", [C, 2, F], BF16, kind="ExternalInput").ap()
    b0 = nc.dram_tensor("b0", [128, 2], F32, kind="ExternalInput").ap()
    b1 = nc.dram_tensor("b1", [128, 2], F32, kind="ExternalInput").ap()
    out = nc.dram_tensor(
        "out", [2, 128, IMG_PER_CORE, GRID], BF16, kind="ExternalOutput"
    ).ap()

    with tile.TileContext(nc) as tc:
        with (
            tc.tile_pool(name="consts", bufs=1) as consts,
            tc.tile_pool(name="xT", bufs=2) as xT,
            tc.tile_pool(name="h1", bufs=4) as h1p,
            tc.tile_pool(name="outb", bufs=8) as outb,
            tc.tile_pool(name="ps1", bufs=5, space="PSUM") as ps1,
            tc.tile_pool(name="ps2", bufs=3, space="PSUM") as ps2,
        ):
            # PE warmup during input DMA: matmuls on a memset tile flip the
            # HAM clock gate to 8/8 before stage 1's first real matmul.
            warm = consts.tile([128, 496], BF16)
            nc.gpsimd.memset(warm[:], 0.0)
            pws = [ps1.tile([128, 496], F32, name="ps1t") for _ in range(2)]
            for i in range(12):
                nc.tensor.matmul(
                    pws[i % 2][:], warm[:, :128], warm[:], start=True, stop=True
                )

            # Critical prologue DMAs split across both HWDGE rings: stage 1's
            # tap loop consumes taps in order, so w0's taps 0-4 (scalar ring)
            # plus x0's first chunk (sync ring, queued first) unblock it, and
            # taps 5-8 arrive while taps 0-4 stream.
            w0b = consts.tile([128, 9, F], BF16)
            w1b = consts.tile([128, 2, F], BF16)
            b0s = consts.tile([128, 2], F32)
            b1s = consts.tile([128, 2], F32)

            # x loads on the sync ring, in image order (the HWDGE ring is
            # FIFO so earlier chunks complete first). Image 0's first chunk
            # is small so stage 1 can start early; bufs=2 defers images 2-3
            # behind stage1 progress via pool-slot reuse.
            def load_x(img, chunks=(2048,), eng=None):
                xt = xT.tile([128, XCOLS], BF16, name="xt")
                c0 = 0
                for c1 in list(chunks) + [XCOLS]:
                    (eng or nc.sync).dma_start(xt[:, c0:c1], x[img, :, c0:c1])
                    c0 = c1
                return xt

            xt0 = xT.tile([128, XCOLS], BF16, name="xt")
            nc.sync.dma_start(xt0[:, :1152], x[0, :, :1152])
            nc.scalar.dma_start(w0b[:, :5], w0[:, :5])
            nc.sync.dma_start(w0b[:, 5:], w0[:, 5:])
            nc.scalar.dma_start(w1b[:], w1)
            nc.scalar.dma_start(b0s[:], b0)
            nc.scalar.dma_start(b1s[:], b1)
            nc.sync.dma_start(xt0[:, 1152:], x[0, :, 1152:])
            xts = [xt0]

            def stage1(img, after_first_group=None):
                xv = xts[img][:].rearrange("p (r c) -> p r c", c=64)
                h1 = []
                for k in range(2):
                    h1.append(h1p.tile([128, GRID], BF16, name="h1t"))
                for h in range(2):
                    for gp in range(4):
                        blks = [2 * gp, 2 * gp + 1]
                        pss = [
                            ps1.tile([128, 496], F32, name="ps1t") for _ in blks
                        ]
                        for t in range(9):
                            di, dj = t // 3, t % 3
                            wtap = w0b[:, t, 128 * h : 128 * (h + 1)]
                            for q, b in enumerate(blks):
                                r0 = 8 * b + di
                                nc.tensor.matmul(
                                    pss[q][:, : BLKN[b]],
                                    wtap,
                                    xv[:, r0 : r0 + ROWS[b], dj : dj + 62],
                                    start=(t == 0),
                                    stop=(t == 8),
                                )
                        for q, b in enumerate(blks):
                            nc.scalar.activation(
                                h1[h][:, BLKS[b] : BLKS[b] + BLKN[b]],
                                pss[q][:, : BLKN[b]],
                                RELU,
                                bias=b0s[:, h : h + 1],
                            )
                        if after_first_group is not None and h == 0 and gp == 0:
                            # emit here so the triggers sit on the scalar
                            # queue behind stage1's first ACTs: the deferred
                            # image's transfer can't compete with the
                            # critical image-0 + weights DMAs.
                            after_first_group()
                            after_first_group = None
                return h1

            ADD = mybir.AluOpType.add
            MAX = mybir.AluOpType.max

            def stage2(img, h1):
                # k-outer groups: one LDWEIGHTS per group of matmuls.
                # Post-processing alternates scalar-ACT and DVE; output DMAs
                # batch two blocks per trigger and alternate rings per half.
                for h in range(2):
                    for gq in range(4):
                        blks = [2 * gq, 2 * gq + 1]
                        pss = [
                            ps2.tile([128, 496], F32, name="ps2t") for _ in blks
                        ]
                        for k in range(2):
                            wk = w1b[:, k, 128 * h : 128 * (h + 1)]
                            for q, b in enumerate(blks):
                                nc.tensor.matmul(
                                    pss[q][:, : BLKN[b]],
                                    wk,
                                    h1[k][:, BLKS[b] : BLKS[b] + BLKN[b]],
                                    start=(k == 0),
                                    stop=(k == 1),
                                )
                        ot = outb.tile([128, 2 * 496], BF16, name="ot")
                        for q, b in enumerate(blks):
                            dst = ot[:, q * 496 :][:, : BLKN[b]]
                            if (gq + q) % 2 == 0:
                                nc.scalar.activation(
                                    dst, pss[q][:, : BLKN[b]], RELU,
                                    bias=b1s[:, h : h + 1],
                                )
                            else:
                                nc.vector.tensor_scalar(
                                    dst, pss[q][:, : BLKN[b]],
                                    b1s[:, h : h + 1], 0.0, ADD, MAX,
                                )
                        ncols = sum(BLKN[b] for b in blks)
                        eng = nc.gpsimd if h == 0 else nc.sync
                        eng.dma_start(
                            out[h, :, img, BLKS[blks[0]] : BLKS[blks[0]] + ncols],
                            ot[:, :ncols],
                        )

            for img in range(IMG_PER_CORE):
                if img == 0:
                    h1 = stage1(
                        0,
                        after_first_group=lambda: xts.append(
                            load_x(1, eng=nc.scalar)
                        ),
                    )
                else:
                    h1 = stage1(img)
                stage2(img, h1)
                if img + 2 < IMG_PER_CORE:
                    xts.append(load_x(img + 2))

    _split_multi_waits(nc)
    return nc


_NC_CACHE = None


def kernel(inputs, w0, b0, w1, b1):
    global _NC_CACHE
    x = np.asarray(inputs, dtype=np.float32)
    w0 = np.asarray(w0, dtype=np.float32)
    w1 = np.asarray(w1, dtype=np.float32)
    b0 = np.asarray(b0, dtype=np.float32)
    b1 = np.asarray(b1, dtype=np.float32)

    if _NC_CACHE is None:
        _NC_CACHE = build_nc()
    nc = _NC_CACHE

    bf = ml_dtypes.bfloat16
    xs = x.reshape(N_CORES, IMG_PER_CORE, HW, C)
    w0h = np.ascontiguousarray(w0.reshape(9, C, F).transpose(1, 0, 2).astype(bf))
    w1h = np.ascontiguousarray(w1.reshape(2, C, F).transpose(1, 0, 2).astype(bf))
    b0h = np.ascontiguousarray(b0.reshape(2, 128).T)
    b1h = np.ascontiguousarray(b1.reshape(2, 128).T)

    in_maps = []
    for c in range(N_CORES):
        xt = np.ascontiguousarray(xs[c].transpose(0, 2, 1).astype(bf))
        in_maps.append({"x": xt, "w0": w0h, "w1": w1h, "b0": b0h, "b1": b1h})

    res = run_bass_kernel_spmd(nc, in_maps, core_ids=list(range(N_CORES)))

    final = np.empty((B, 62, 62, F), np.float32)
    vf = final.reshape(F, 62 * 62, B)  # the [F, N, B] view the reference reshapes
    for c in range(N_CORES):
        oc = res.results[c]["out"].astype(np.float32).reshape(F, IMG_PER_CORE, GRID)
        for i in range(IMG_PER_CORE):
            vf[:, :, c * IMG_PER_CORE + i] = oc[:, i]
    return final
